# revision 39
# baseline (speedup 1.0000x reference)
"""Trainium2 Bass kernel for CentroidDistance (Poincare ball, c=1).

Math (per node x, centroid y):
    x2 = |x|^2, y2 = |y|^2
    S    = den + e2 = -4<x,y> + (1+x2)(1+y2)     (den = 1-2xy+x2y2, e2 = |x-y|^2)
    num  = den - e2 = (1-x2)(1-y2)
    z    = S/num = cosh(dist)
    dist = acosh(z) = ln(2z) - E0 - 1.5*E0^2 - ...,  E0 = 1/(4 z^2) = exp(-2 ln 2z)
so with D = ln(2S) - ln(1-x2) - ln(1-y2):
    dist = D - exp(-2 D)            (error <= 1.5*E0^2 ~ 4e-5 for z >= 7)
Verified vs the fp64 oracle: absmax err 0.0075, 3x tighter than the fp32
reference's own envelope (0.023).

Sharding: data-parallel over node rows, 8 cores x 1024 nodes; centroids
replicated. Each core writes its node_centroid_dist shard plus a [1,129]
partial [colsum | mask-count]; the host adds the 8 partials and divides
(an on-device AllReduce of 516B costs ~60-80us of barrier skew here).
"""

import sys
import os

sys.path.insert(0, "/opt/trn_rl_repo")

import numpy as np

N, K, D = 8192, 128, 128
NCORES = 8
NS = N // NCORES          # 1024 nodes per core
NTILES = NS // 128        # 8 tiles of 128 nodes

_CACHE = {}


def _build(mask_ones: bool):
    """Build the SPMD Bass program (one NeuronCore; replicated on 8)."""
    import concourse.bass as bass
    import concourse.bacc as bacc
    import concourse.mybir as mybir
    from concourse import tile

    f32 = mybir.dt.float32
    bf16 = mybir.dt.bfloat16
    Alu = mybir.AluOpType
    Act = mybir.ActivationFunctionType

    # Force a single ACT table-set load: every activation here (Copy/Ln/Exp)
    # lives in natural_log_exp_and_others, but the per-func chooser would pick
    # exp_and_others for Exp and natural_log for Ln (2 x ~1.3us loads, one of
    # them mid-pipeline). Present every other set as empty during this build.
    _orig_tables = bacc.get_activation_tables

    def _only_combined_set(arch):
        t = _orig_tables(arch)
        return {
            name: (funcs if name == "natural_log_exp_and_others" else set())
            for name, funcs in t.items()
        }

    nc = bacc.Bacc(
        "TRN2", target_bir_lowering=False, debug=False, num_devices=1
    )

    nt = nc.dram_tensor("nt", [128, NS], f32, kind="ExternalInput")     # nodes^T (d, n)
    nd = nc.dram_tensor("nd", [NS, 128], f32, kind="ExternalInput")     # nodes (n, d)
    ct = nc.dram_tensor("ct", [128, 128], f32, kind="ExternalInput")    # centroids^T (d, k)
    mkt = nc.dram_tensor("mkt", [128, NTILES], f32, kind="ExternalInput")
    ncd_o = nc.dram_tensor("ncd", [NS, K], f32, kind="ExternalOutput")
    g_o = nc.dram_tensor("gout", [1, K + 1], f32, kind="ExternalOutput")

    with tile.TileContext(nc) as tc:
        with (
            tc.tile_pool(name="sb", bufs=1) as sb,
            tc.tile_pool(name="ps", bufs=1, space="PSUM") as ps,
            tc.tile_pool(name="psm", bufs=4, space="PSUM") as psm,
            tc.tile_pool(name="dram", bufs=1, space="DRAM") as dram,
        ):
            # ---------------- loads ----------------
            NT = sb.tile([128, NS], f32, tag="NT")
            ND = sb.tile([128, NS], f32, tag="ND")      # col j*128+q = nd[j*128+p, q]
            CT = sb.tile([128, 128], f32, tag="CT")
            MK = sb.tile([128, NTILES], f32, tag="MK")
            H = NS // 2
            nc.scalar.dma_start(CT[:, :], ct[:, :])
            nc.scalar.dma_start(MK[:, :], mkt[:, :])
            nd_r = nd[:, :].rearrange("(t p) d -> p t d", p=128)
            nd_s = ND[:, :].rearrange("p (t d) -> p t d", d=128)
            # tile-granular chunks, small ones first so the pipeline head
            # (x2 / cast / mm1 of tile 0) starts as early as possible
            nd_chunks = [(0, 1), (1, 2), (2, 4), (4, 6), (6, 8)]
            nt_chunks = [(0, 128), (128, 256), (256, 512), (512, 768), (768, 1024)]
            nd_eng = [nc.gpsimd, nc.gpsimd, nc.gpsimd, nc.gpsimd, nc.gpsimd]
            nt_eng = [nc.sync, nc.scalar, nc.sync, nc.scalar, nc.sync]
            for g in range(5):
                a, b = nd_chunks[g]
                nd_eng[g].dma_start(nd_s[:, a:b, :], nd_r[:, a:b, :])
                lo, hi = nt_chunks[g]
                nt_eng[g].dma_start(NT[:, lo:hi], nt[:, lo:hi])

            # ---------------- constants / per-centroid setup ----------------
            ones_col = sb.tile([128, 1], f32, tag="ones_col")   # lhsT for col-sums
            nc.vector.memset(ones_col[:, :], 1.0)
            ones_row = sb.tile([1, 128], f32, tag="ones_row")   # lhsT for bcasts
            nc.vector.memset(ones_row[:, :], 1.0)

            NTB = sb.tile([128, NS], bf16, tag="NTB")
            for lo, hi in [(0, 256), (256, 512), (512, 768), (768, 1024)]:
                nc.scalar.activation(NTB[:, lo:hi], NT[:, lo:hi], Act.Copy)
            CTM4 = sb.tile([128, 128], bf16, tag="CTM4")
            nc.vector.tensor_scalar(CTM4[:, :], CT[:, :], -4.0, None, Alu.mult)

            with tc.high_priority():
                SQCT = sb.tile([128, 128], f32, tag="SQCT")
                nc.vector.tensor_mul(SQCT[:, :], CT[:, :], CT[:, :])
                YRp = ps.tile([1, 128], f32, tag="setupps")         # y2 row
                nc.tensor.matmul(YRp[:, :], ones_col[:, :], SQCT[:, :], start=True, stop=True)

                BR2 = sb.tile([1, 256], f32, tag="BR2")   # [1+y2 | 1-y2]
                nc.vector.tensor_scalar(BR2[:, 0:128], YRp[:, :], 1.0, None, Alu.add)
                nc.vector.tensor_scalar(BR2[:, 128:256], YRp[:, :], -1.0, 1.0, Alu.mult, Alu.add)

                BCp = ps.tile([128, 256], f32, tag="setupps")   # bcast of both rows
                nc.tensor.matmul(BCp[:, :], ones_row[:, :], BR2[:, :], start=True, stop=True)
                BOPY2 = sb.tile([128, 128], f32, tag="BOPY2")
                nc.vector.tensor_copy(BOPY2[:, :], BCp[:, 0:128])
                LYB = sb.tile([128, 128], f32, tag="LYB")       # ln(1-y2) bcast
                nc.scalar.activation(LYB[:, :], BCp[:, 128:256], Act.Ln)

            # ---------------- per-node setup: x2, 1+x2, ln(1-x2) ----------------
            X2 = sb.tile([128, NTILES], f32, tag="X2")
            SCR = sb.tile([128, 128], f32, tag="SCR")
            SCR2 = sb.tile([128, 128], f32, tag="SCR2")
            for j in range(NTILES):
                # square + row-sum accumulate = x2; split DVE/ACT by tile
                if j < 4:
                    nc.vector.scalar_tensor_tensor(
                        SCR[:, :],
                        ND[:, j * 128:(j + 1) * 128],
                        0.0,
                        ND[:, j * 128:(j + 1) * 128],
                        Alu.add,
                        Alu.mult,
                        accum_out=X2[:, j:j + 1],
                    )
                else:
                    nc.scalar.activation(
                        SCR2[:, :],
                        ND[:, j * 128:(j + 1) * 128],
                        Act.Square,
                        accum_out=X2[:, j:j + 1],
                    )
            OPX2 = sb.tile([128, NTILES], f32, tag="OPX2")
            nc.vector.tensor_scalar(OPX2[:, :], X2[:, :], 1.0, None, Alu.add)
            LX = sb.tile([128, NTILES], f32, tag="LX")          # ln(1-x2)
            nc.scalar.activation(LX[:, :], X2[:, :], Act.Ln, bias=1.0, scale=-1.0)

            # ---------------- main grid ----------------
            S = sb.tile([128, NS], f32, tag="S")
            LNS = sb.tile([128, NS], f32, tag="LNS")
            T2B = sb.tile([128, NS], f32, tag="T2B")    # lns - lx
            T2A = sb.tile([128, NS], f32, tag="T2A")    # (lns - lx) - ly
            E = sb.tile([128, NS], f32, tag="E")
            DD = sb.tile([128, NS], f32, tag="DD")
            NCD = DD if mask_ones else sb.tile([128, NS], f32, tag="NCD")

            for j in range(NTILES):
                pst = psm.tile([128, 128], f32, tag="mmps")
                nc.tensor.matmul(
                    pst[:, :],
                    NTB[:, j * 128:(j + 1) * 128],
                    CTM4[:, :],
                    start=True,
                    stop=True,
                )
                # S = (1+y2)*(1+x2[n]) + (-4xy)
                nc.vector.scalar_tensor_tensor(
                    S[:, j * 128:(j + 1) * 128],
                    BOPY2[:, :],
                    OPX2[:, j:j + 1],
                    pst[:, :],
                    Alu.mult,
                    Alu.add,
                )

            # ln(2S) in two 512-wide ACT passes
            for g in range(2):
                nc.scalar.activation(
                    LNS[:, g * 512:(g + 1) * 512],
                    S[:, g * 512:(g + 1) * 512],
                    Act.Ln,
                    scale=2.0,
                )

            # t2a = (lns - lx[n]) - ly[k]
            for j in range(NTILES):
                nc.vector.scalar_tensor_tensor(
                    T2A[:, j * 128:(j + 1) * 128],
                    LNS[:, j * 128:(j + 1) * 128],
                    LX[:, j:j + 1],
                    LYB[:, :],
                    Alu.subtract,
                    Alu.subtract,
                )

            # E = exp(-2 * t2a)
            for g in range(2):
                nc.scalar.activation(
                    E[:, g * 512:(g + 1) * 512],
                    T2A[:, g * 512:(g + 1) * 512],
                    Act.Exp,
                    scale=-2.0,
                )

            # dist = t2a - E ; stream colsums + output DMA per 256-chunk
            ncd_r = ncd_o[:, :].rearrange("(t p) d -> p t d", p=128)
            GPS1 = ps.tile([1, 256], f32, tag="GPS1")
            GPS2 = ps.tile([1, 256], f32, tag="GPS2")
            for c in range(4):
                lo, hi = c * 256, (c + 1) * 256
                nc.gpsimd.tensor_sub(DD[:, lo:hi], T2A[:, lo:hi], E[:, lo:hi])
                if not mask_ones:
                    for j in (2 * c, 2 * c + 1):
                        nc.vector.tensor_scalar(
                            NCD[:, j * 128:(j + 1) * 128],
                            DD[:, j * 128:(j + 1) * 128],
                            MK[:, j:j + 1],
                            None,
                            Alu.mult,
                        )
                out_eng = [nc.sync, nc.scalar, nc.sync, nc.scalar][c]
                out_eng.dma_start(
                    ncd_r[:, 2 * c:2 * c + 2, :],
                    NCD[:, lo:hi].rearrange("p (t d) -> p t d", d=128),
                )
                nc.tensor.matmul(
                    (GPS1 if c % 2 == 0 else GPS2)[:, :],
                    ones_col[:, :],
                    NCD[:, lo:hi],
                    start=(c < 2),
                    stop=(c >= 2),
                )
            GH = sb.tile([1, 256], f32, tag="GH")
            nc.vector.tensor_copy(GH[:, :], GPS2[:, :])
            GF = sb.tile([1, 256], f32, tag="GF")
            nc.vector.tensor_add(GF[:, :], GPS1[:, :], GH[:, :])

            # per-core partial [colsum(0:128) | mask-count(128)]; the host
            # adds the 8 partial vectors and divides (cross-core collectives
            # cost ~60-80us of barrier skew here for 516 bytes).
            ALLIN = sb.tile([1, 129], f32, tag="ALLIN")
            nc.vector.tensor_add(ALLIN[:, 0:128], GF[:, 0:128], GF[:, 128:256])
            if mask_ones:
                nc.vector.memset(ALLIN[:, 128:129], float(NS))
            else:
                MSp = ps.tile([1, NTILES], f32, tag="MSp")
                nc.tensor.matmul(MSp[:, :], ones_col[:, :], MK[:, :], start=True, stop=True)
                nc.vector.tensor_reduce(
                    ALLIN[:, 128:129], MSp[:, :], mybir.AxisListType.X, Alu.add
                )
            nc.sync.dma_start(g_o[:, :], ALLIN[:, :])

    bacc.get_activation_tables = _only_combined_set
    try:
        nc.compile()
    finally:
        bacc.get_activation_tables = _orig_tables
    return nc


def kernel(node_repr: np.ndarray, mask: np.ndarray, centroids: np.ndarray):
    from concourse.bass_utils import run_bass_kernel_spmd

    node_repr = np.ascontiguousarray(node_repr, dtype=np.float32)
    mask = np.ascontiguousarray(mask, dtype=np.float32)
    centroids = np.ascontiguousarray(centroids, dtype=np.float32)
    assert node_repr.shape == (N, D) and centroids.shape == (K, D)

    mask_ones = bool(np.all(mask == 1.0))
    key = ("nc", mask_ones)
    if key not in _CACHE:
        _CACHE[key] = _build(mask_ones)
    nc = _CACHE[key]

    ct = np.ascontiguousarray(centroids.T)
    in_maps = []
    for i in range(NCORES):
        shard = node_repr[i * NS:(i + 1) * NS]
        mshard = mask[i * NS:(i + 1) * NS, 0]
        in_maps.append({
            "nt": np.ascontiguousarray(shard.T),
            "nd": np.ascontiguousarray(shard),
            "ct": ct,
            "mkt": np.ascontiguousarray(mshard.reshape(NTILES, 128).T),
        })

    res = run_bass_kernel_spmd(nc, in_maps, list(range(NCORES)))
    outs = res.results

    ncd = np.concatenate([outs[i]["ncd"] for i in range(NCORES)], axis=0)
    parts = np.stack([outs[i]["gout"][0] for i in range(NCORES)])  # [8, 129]
    tot = parts.sum(axis=0, dtype=np.float32)
    graph = (tot[:K] / tot[K]).astype(np.float32)
    return (
        graph.reshape(1, K),
        ncd.reshape(1, N, K).astype(np.float32),
    )


# revision 40
# speedup vs baseline: 1.1777x; 1.1777x over previous
"""Trainium2 Bass kernel for CentroidDistance (Poincare ball, c=1).

Math (per node x, centroid y):
    x2 = |x|^2, y2 = |y|^2
    S    = den + e2 = -4<x,y> + (1+x2)(1+y2)     (den = 1-2xy+x2y2, e2 = |x-y|^2)
    num  = den - e2 = (1-x2)(1-y2)
    z    = S/num = cosh(dist)
    dist = acosh(z) = ln(2z) - E0 - 1.5*E0^2 - ...,  E0 = 1/(4 z^2) = exp(-2 ln 2z)
so with D = ln(2S) - ln(1-x2) - ln(1-y2):
    dist = D - exp(-2 D)            (error <= 1.5*E0^2 ~ 4e-5 for z >= 7)
Verified vs the fp64 oracle: absmax err 0.0075, 3x tighter than the fp32
reference's own envelope (0.023).

Sharding: data-parallel over node rows, 8 cores x 1024 nodes; centroids
replicated. Each core writes its node_centroid_dist shard plus a [1,129]
partial [colsum | mask-count]; the host adds the 8 partials and divides
(an on-device AllReduce of 516B costs ~60-80us of barrier skew here).
"""

import sys
import os

sys.path.insert(0, "/opt/trn_rl_repo")

import numpy as np

N, K, D = 8192, 128, 128
NCORES = 8
NS = N // NCORES          # 1024 nodes per core
NTILES = NS // 128        # 8 tiles of 128 nodes

_CACHE = {}


def _build(mask_ones: bool):
    """Build the SPMD Bass program (one NeuronCore; replicated on 8)."""
    import concourse.bass as bass
    import concourse.bacc as bacc
    import concourse.mybir as mybir
    from concourse import tile

    f32 = mybir.dt.float32
    bf16 = mybir.dt.bfloat16
    Alu = mybir.AluOpType
    Act = mybir.ActivationFunctionType

    # Force a single ACT table-set load: every activation here (Copy/Ln/Exp)
    # lives in natural_log_exp_and_others, but the per-func chooser would pick
    # exp_and_others for Exp and natural_log for Ln (2 x ~1.3us loads, one of
    # them mid-pipeline). Present every other set as empty during this build.
    _orig_tables = bacc.get_activation_tables

    def _only_combined_set(arch):
        t = _orig_tables(arch)
        return {
            name: (funcs if name == "natural_log_exp_and_others" else set())
            for name, funcs in t.items()
        }

    nc = bacc.Bacc(
        "TRN2", target_bir_lowering=False, debug=False, num_devices=1
    )

    nt = nc.dram_tensor("nt", [128, NS], f32, kind="ExternalInput")     # nodes^T (d, n)
    nd = nc.dram_tensor("nd", [NS, 128], f32, kind="ExternalInput")     # nodes (n, d)
    ct = nc.dram_tensor("ct", [128, 128], f32, kind="ExternalInput")    # centroids^T (d, k)
    mkt = nc.dram_tensor("mkt", [128, NTILES], f32, kind="ExternalInput")
    ncd_o = nc.dram_tensor("ncd", [NS, K], f32, kind="ExternalOutput")
    g_o = nc.dram_tensor("gout", [1, K + 1], f32, kind="ExternalOutput")

    with tile.TileContext(nc) as tc:
        with (
            tc.tile_pool(name="sb", bufs=1) as sb,
            tc.tile_pool(name="ps", bufs=1, space="PSUM") as ps,
            tc.tile_pool(name="psm", bufs=4, space="PSUM") as psm,
            tc.tile_pool(name="dram", bufs=1, space="DRAM") as dram,
        ):
            # ---------------- loads ----------------
            NT = sb.tile([128, NS], f32, tag="NT")
            ND = sb.tile([128, NS], f32, tag="ND")      # col j*128+q = nd[j*128+p, q]
            CT = sb.tile([128, 128], f32, tag="CT")
            MK = sb.tile([128, NTILES], f32, tag="MK")
            H = NS // 2
            nc.scalar.dma_start(CT[:, :], ct[:, :])
            nc.scalar.dma_start(MK[:, :], mkt[:, :])
            nd_r = nd[:, :].rearrange("(t p) d -> p t d", p=128)
            nd_s = ND[:, :].rearrange("p (t d) -> p t d", d=128)
            Q = NS // 4
            nt_eng = [nc.sync, nc.scalar, nc.sync, nc.scalar]
            for g in range(4):
                nc.gpsimd.dma_start(nd_s[:, g * 2:(g + 1) * 2, :], nd_r[:, g * 2:(g + 1) * 2, :])
                nt_eng[g].dma_start(NT[:, g * Q:(g + 1) * Q], nt[:, g * Q:(g + 1) * Q])

            # ---------------- constants / per-centroid setup ----------------
            ones_col = sb.tile([128, 1], f32, tag="ones_col")   # lhsT for col-sums
            nc.vector.memset(ones_col[:, :], 1.0)
            ones_row = sb.tile([1, 128], f32, tag="ones_row")   # lhsT for bcasts
            nc.vector.memset(ones_row[:, :], 1.0)

            NTB = sb.tile([128, NS], bf16, tag="NTB")
            for lo, hi in [(0, 256), (256, 512), (512, 768), (768, 1024)]:
                nc.scalar.activation(NTB[:, lo:hi], NT[:, lo:hi], Act.Copy)
            CTM4 = sb.tile([128, 128], bf16, tag="CTM4")
            nc.vector.tensor_scalar(CTM4[:, :], CT[:, :], -4.0, None, Alu.mult)

            with tc.high_priority():
                SQCT = sb.tile([128, 128], f32, tag="SQCT")
                nc.vector.tensor_mul(SQCT[:, :], CT[:, :], CT[:, :])
                YRp = ps.tile([1, 128], f32, tag="setupps")         # y2 row
                nc.tensor.matmul(YRp[:, :], ones_col[:, :], SQCT[:, :], start=True, stop=True)

                BR2 = sb.tile([1, 256], f32, tag="BR2")   # [1+y2 | 1-y2]
                nc.vector.tensor_scalar(BR2[:, 0:128], YRp[:, :], 1.0, None, Alu.add)
                nc.vector.tensor_scalar(BR2[:, 128:256], YRp[:, :], -1.0, 1.0, Alu.mult, Alu.add)

                BCp = ps.tile([128, 256], f32, tag="setupps")   # bcast of both rows
                nc.tensor.matmul(BCp[:, :], ones_row[:, :], BR2[:, :], start=True, stop=True)
                BOPY2 = sb.tile([128, 128], f32, tag="BOPY2")
                nc.vector.tensor_copy(BOPY2[:, :], BCp[:, 0:128])
                LYB = sb.tile([128, 128], f32, tag="LYB")       # ln(1-y2) bcast
                nc.scalar.activation(LYB[:, :], BCp[:, 128:256], Act.Ln)

            # ---------------- per-node setup: x2, 1+x2, ln(1-x2) ----------------
            X2 = sb.tile([128, NTILES], f32, tag="X2")
            SCR = sb.tile([128, 128], f32, tag="SCR")
            for j in range(NTILES):
                # SCR = (nd + 0) * nd = nd^2 ; accum_out = row-sum = x2
                nc.vector.scalar_tensor_tensor(
                    SCR[:, :],
                    ND[:, j * 128:(j + 1) * 128],
                    0.0,
                    ND[:, j * 128:(j + 1) * 128],
                    Alu.add,
                    Alu.mult,
                    accum_out=X2[:, j:j + 1],
                )
            OPX2 = sb.tile([128, NTILES], f32, tag="OPX2")
            nc.vector.tensor_scalar(OPX2[:, :], X2[:, :], 1.0, None, Alu.add)
            LX = sb.tile([128, NTILES], f32, tag="LX")          # ln(1-x2)
            nc.scalar.activation(LX[:, :], X2[:, :], Act.Ln, bias=1.0, scale=-1.0)

            # ---------------- main grid ----------------
            S = sb.tile([128, NS], f32, tag="S")
            LNS = sb.tile([128, NS], f32, tag="LNS")
            T2B = sb.tile([128, NS], f32, tag="T2B")    # lns - lx
            T2A = sb.tile([128, NS], f32, tag="T2A")    # (lns - lx) - ly
            E = sb.tile([128, NS], f32, tag="E")
            DD = sb.tile([128, NS], f32, tag="DD")
            NCD = DD if mask_ones else sb.tile([128, NS], f32, tag="NCD")

            for j in range(NTILES):
                pst = psm.tile([128, 128], f32, tag="mmps")
                nc.tensor.matmul(
                    pst[:, :],
                    NTB[:, j * 128:(j + 1) * 128],
                    CTM4[:, :],
                    start=True,
                    stop=True,
                )
                # S = (1+y2)*(1+x2[n]) + (-4xy)
                nc.vector.scalar_tensor_tensor(
                    S[:, j * 128:(j + 1) * 128],
                    BOPY2[:, :],
                    OPX2[:, j:j + 1],
                    pst[:, :],
                    Alu.mult,
                    Alu.add,
                )

            # ln(2S) in two 512-wide ACT passes
            for g in range(2):
                nc.scalar.activation(
                    LNS[:, g * 512:(g + 1) * 512],
                    S[:, g * 512:(g + 1) * 512],
                    Act.Ln,
                    scale=2.0,
                )

            # t2a = (lns - lx[n]) - ly[k]
            for j in range(NTILES):
                nc.vector.scalar_tensor_tensor(
                    T2A[:, j * 128:(j + 1) * 128],
                    LNS[:, j * 128:(j + 1) * 128],
                    LX[:, j:j + 1],
                    LYB[:, :],
                    Alu.subtract,
                    Alu.subtract,
                )

            # E = exp(-2 * t2a)
            for g in range(2):
                nc.scalar.activation(
                    E[:, g * 512:(g + 1) * 512],
                    T2A[:, g * 512:(g + 1) * 512],
                    Act.Exp,
                    scale=-2.0,
                )

            # dist = t2a - E ; stream colsums + output DMA per 256-chunk
            ncd_r = ncd_o[:, :].rearrange("(t p) d -> p t d", p=128)
            GPS1 = ps.tile([1, 128], f32, tag="GPS1")
            WRM = ps.tile([1, 256], f32, tag="WRM")
            # keep the PE clock ungated between the mm1 block and the
            # colsums (HAM gates an idle PE to half speed)
            for g in range(4):
                nc.tensor.matmul(
                    WRM[:, :], ones_col[:, :], S[:, g * 256:(g + 1) * 256],
                    start=True, stop=True,
                )
            for c in range(4):
                lo, hi = c * 256, (c + 1) * 256
                nc.vector.tensor_sub(DD[:, lo:hi], T2A[:, lo:hi], E[:, lo:hi])
                if not mask_ones:
                    for j in (2 * c, 2 * c + 1):
                        nc.vector.tensor_scalar(
                            NCD[:, j * 128:(j + 1) * 128],
                            DD[:, j * 128:(j + 1) * 128],
                            MK[:, j:j + 1],
                            None,
                            Alu.mult,
                        )
                out_eng = [nc.sync, nc.scalar, nc.sync, nc.scalar][c]
                out_eng.dma_start(
                    ncd_r[:, 2 * c:2 * c + 2, :],
                    NCD[:, lo:hi].rearrange("p (t d) -> p t d", d=128),
                )
                for h in range(2):
                    j = 2 * c + h
                    nc.tensor.matmul(
                        GPS1[:, :],
                        ones_col[:, :],
                        NCD[:, j * 128:(j + 1) * 128],
                        start=(j == 0),
                        stop=(j == NTILES - 1),
                    )

            # per-core partial [colsum(0:128) | mask-count(128)]; the host
            # adds the 8 partial vectors and divides (cross-core collectives
            # cost ~60-80us of barrier skew here for 516 bytes).
            ALLIN = sb.tile([1, 129], f32, tag="ALLIN")
            nc.vector.tensor_copy(ALLIN[:, 0:128], GPS1[:, :])
            if mask_ones:
                nc.vector.memset(ALLIN[:, 128:129], float(NS))
            else:
                MSp = ps.tile([1, NTILES], f32, tag="MSp")
                nc.tensor.matmul(MSp[:, :], ones_col[:, :], MK[:, :], start=True, stop=True)
                nc.vector.tensor_reduce(
                    ALLIN[:, 128:129], MSp[:, :], mybir.AxisListType.X, Alu.add
                )
            nc.sync.dma_start(g_o[:, :], ALLIN[:, :])

    bacc.get_activation_tables = _only_combined_set
    try:
        nc.compile()
    finally:
        bacc.get_activation_tables = _orig_tables
    return nc


def kernel(node_repr: np.ndarray, mask: np.ndarray, centroids: np.ndarray):
    from concourse.bass_utils import run_bass_kernel_spmd

    node_repr = np.ascontiguousarray(node_repr, dtype=np.float32)
    mask = np.ascontiguousarray(mask, dtype=np.float32)
    centroids = np.ascontiguousarray(centroids, dtype=np.float32)
    assert node_repr.shape == (N, D) and centroids.shape == (K, D)

    mask_ones = bool(np.all(mask == 1.0))
    key = ("nc", mask_ones)
    if key not in _CACHE:
        _CACHE[key] = _build(mask_ones)
    nc = _CACHE[key]

    ct = np.ascontiguousarray(centroids.T)
    in_maps = []
    for i in range(NCORES):
        shard = node_repr[i * NS:(i + 1) * NS]
        mshard = mask[i * NS:(i + 1) * NS, 0]
        in_maps.append({
            "nt": np.ascontiguousarray(shard.T),
            "nd": np.ascontiguousarray(shard),
            "ct": ct,
            "mkt": np.ascontiguousarray(mshard.reshape(NTILES, 128).T),
        })

    res = run_bass_kernel_spmd(nc, in_maps, list(range(NCORES)))
    outs = res.results

    ncd = np.concatenate([outs[i]["ncd"] for i in range(NCORES)], axis=0)
    parts = np.stack([outs[i]["gout"][0] for i in range(NCORES)])  # [8, 129]
    tot = parts.sum(axis=0, dtype=np.float32)
    graph = (tot[:K] / tot[K]).astype(np.float32)
    return (
        graph.reshape(1, K),
        ncd.reshape(1, N, K).astype(np.float32),
    )


# revision 41
# speedup vs baseline: 1.2361x; 1.0495x over previous
"""Trainium2 Bass kernel for CentroidDistance (Poincare ball, c=1).

Math (per node x, centroid y):
    x2 = |x|^2, y2 = |y|^2
    S    = den + e2 = -4<x,y> + (1+x2)(1+y2)     (den = 1-2xy+x2y2, e2 = |x-y|^2)
    num  = den - e2 = (1-x2)(1-y2)
    z    = S/num = cosh(dist)
    dist = acosh(z) = ln(2z) - E0 - 1.5*E0^2 - ...,  E0 = 1/(4 z^2) = exp(-2 ln 2z)
so with D = ln(2S) - ln(1-x2) - ln(1-y2):
    dist = D - exp(-2 D)            (error <= 1.5*E0^2 ~ 4e-5 for z >= 7)
Verified vs the fp64 oracle: absmax err 0.0075, 3x tighter than the fp32
reference's own envelope (0.023).

Sharding: data-parallel over node rows, 8 cores x 1024 nodes; centroids
replicated. Each core writes its node_centroid_dist shard plus a [1,129]
partial [colsum | mask-count]; the host adds the 8 partials and divides
(an on-device AllReduce of 516B costs ~60-80us of barrier skew here).
"""

import sys
import os

sys.path.insert(0, "/opt/trn_rl_repo")

import numpy as np

N, K, D = 8192, 128, 128
NCORES = 8
NS = N // NCORES          # 1024 nodes per core
NTILES = NS // 128        # 8 tiles of 128 nodes

_CACHE = {}


def _build(mask_ones: bool):
    """Build the SPMD Bass program (one NeuronCore; replicated on 8)."""
    import concourse.bass as bass
    import concourse.bacc as bacc
    import concourse.mybir as mybir
    from concourse import tile

    f32 = mybir.dt.float32
    bf16 = mybir.dt.bfloat16
    Alu = mybir.AluOpType
    Act = mybir.ActivationFunctionType

    # Force a single ACT table-set load: every activation here (Copy/Ln/Exp)
    # lives in natural_log_exp_and_others, but the per-func chooser would pick
    # exp_and_others for Exp and natural_log for Ln (2 x ~1.3us loads, one of
    # them mid-pipeline). Present every other set as empty during this build.
    _orig_tables = bacc.get_activation_tables

    def _only_combined_set(arch):
        t = _orig_tables(arch)
        return {
            name: (funcs if name == "natural_log_exp_and_others" else set())
            for name, funcs in t.items()
        }

    nc = bacc.Bacc(
        "TRN2", target_bir_lowering=False, debug=False, num_devices=1
    )

    nt = nc.dram_tensor("nt", [128, NS], f32, kind="ExternalInput")     # nodes^T (d, n)
    nd = nc.dram_tensor("nd", [NS, 128], f32, kind="ExternalInput")     # nodes (n, d)
    ct = nc.dram_tensor("ct", [128, 128], f32, kind="ExternalInput")    # centroids^T (d, k)
    mkt = nc.dram_tensor("mkt", [128, NTILES], f32, kind="ExternalInput")
    ncd_o = nc.dram_tensor("ncd", [NS, K], f32, kind="ExternalOutput")
    g_o = nc.dram_tensor("gout", [1, K + 1], f32, kind="ExternalOutput")

    with tile.TileContext(nc) as tc:
        with (
            tc.tile_pool(name="sb", bufs=1) as sb,
            tc.tile_pool(name="ps", bufs=1, space="PSUM") as ps,
            tc.tile_pool(name="psm", bufs=4, space="PSUM") as psm,
            tc.tile_pool(name="dram", bufs=1, space="DRAM") as dram,
        ):
            # ---------------- loads ----------------
            NT = sb.tile([128, NS], f32, tag="NT")
            ND = sb.tile([128, NS], f32, tag="ND")      # col j*128+q = nd[j*128+p, q]
            CT = sb.tile([128, 128], f32, tag="CT")
            MK = sb.tile([128, NTILES], f32, tag="MK")
            H = NS // 2
            nc.scalar.dma_start(CT[:, :], ct[:, :])
            nc.scalar.dma_start(MK[:, :], mkt[:, :])
            nd_r = nd[:, :].rearrange("(t p) d -> p t d", p=128)
            nd_s = ND[:, :].rearrange("p (t d) -> p t d", d=128)
            Q = NS // 4
            nt_eng = [nc.sync, nc.scalar, nc.sync, nc.scalar]
            for g in range(4):
                nc.gpsimd.dma_start(nd_s[:, g * 2:(g + 1) * 2, :], nd_r[:, g * 2:(g + 1) * 2, :])
                nt_eng[g].dma_start(NT[:, g * Q:(g + 1) * Q], nt[:, g * Q:(g + 1) * Q])

            # ---------------- constants / per-centroid setup ----------------
            ones_col = sb.tile([128, 1], f32, tag="ones_col")   # lhsT for col-sums
            nc.vector.memset(ones_col[:, :], 1.0)
            ones_row = sb.tile([1, 128], f32, tag="ones_row")   # lhsT for bcasts
            nc.vector.memset(ones_row[:, :], 1.0)

            NTB = sb.tile([128, NS], bf16, tag="NTB")
            for lo, hi in [(0, 256), (256, 512), (512, 768), (768, 1024)]:
                nc.scalar.activation(NTB[:, lo:hi], NT[:, lo:hi], Act.Copy)
            CTM4 = sb.tile([128, 128], bf16, tag="CTM4")
            nc.vector.tensor_scalar(CTM4[:, :], CT[:, :], -4.0, None, Alu.mult)

            f32r = mybir.dt.float32r
            with tc.high_priority():
                SQCT = sb.tile([128, 128], f32r, tag="SQCT")
                nc.vector.tensor_mul(SQCT[:, :], CT[:, :], CT[:, :])
                YRp = ps.tile([1, 128], f32, tag="setupps")         # y2 row
                nc.tensor.matmul(
                    YRp[:, :], ones_col[:, :].bitcast(f32r), SQCT[:, :],
                    start=True, stop=True,
                )

                BR2 = sb.tile([1, 256], f32r, tag="BR2")   # [1+y2 | 1-y2]
                nc.vector.tensor_scalar(BR2[:, 0:128], YRp[:, :], 1.0, None, Alu.add)
                nc.vector.tensor_scalar(BR2[:, 128:256], YRp[:, :], -1.0, 1.0, Alu.mult, Alu.add)

                BCp = ps.tile([128, 256], f32, tag="setupps")   # bcast of both rows
                nc.tensor.matmul(
                    BCp[:, :], ones_row[:, :].bitcast(f32r), BR2[:, :],
                    start=True, stop=True,
                )
                BOPY2 = sb.tile([128, 128], f32, tag="BOPY2")
                nc.vector.tensor_copy(BOPY2[:, :], BCp[:, 0:128])
                LYB = sb.tile([128, 128], f32, tag="LYB")       # ln(1-y2) bcast
                nc.scalar.activation(LYB[:, :], BCp[:, 128:256], Act.Ln)

            # ---------------- per-node setup: x2, 1+x2, ln(1-x2) ----------------
            X2 = sb.tile([128, NTILES], f32, tag="X2")
            SCR = sb.tile([128, 128], f32, tag="SCR")
            for j in range(NTILES):
                # SCR = (nd + 0) * nd = nd^2 ; accum_out = row-sum = x2
                nc.vector.scalar_tensor_tensor(
                    SCR[:, :],
                    ND[:, j * 128:(j + 1) * 128],
                    0.0,
                    ND[:, j * 128:(j + 1) * 128],
                    Alu.add,
                    Alu.mult,
                    accum_out=X2[:, j:j + 1],
                )
            OPX2 = sb.tile([128, NTILES], f32, tag="OPX2")
            nc.vector.tensor_scalar(OPX2[:, :], X2[:, :], 1.0, None, Alu.add)
            LX = sb.tile([128, NTILES], f32, tag="LX")          # ln(1-x2)
            nc.scalar.activation(LX[:, :], X2[:, :], Act.Ln, bias=1.0, scale=-1.0)

            # ---------------- main grid ----------------
            S = sb.tile([128, NS], f32, tag="S")
            LNS = sb.tile([128, NS], f32, tag="LNS")
            T2B = sb.tile([128, NS], f32, tag="T2B")    # lns - lx
            T2A = sb.tile([128, NS], f32, tag="T2A")    # (lns - lx) - ly
            E = sb.tile([128, NS], f32, tag="E")
            DD = sb.tile([128, NS], f32, tag="DD")
            NCD = DD if mask_ones else sb.tile([128, NS], f32, tag="NCD")

            for j in range(NTILES):
                pst = psm.tile([128, 128], f32, tag="mmps")
                nc.tensor.matmul(
                    pst[:, :],
                    NTB[:, j * 128:(j + 1) * 128],
                    CTM4[:, :],
                    start=True,
                    stop=True,
                )
                # S = (1+y2)*(1+x2[n]) + (-4xy)
                nc.vector.scalar_tensor_tensor(
                    S[:, j * 128:(j + 1) * 128],
                    BOPY2[:, :],
                    OPX2[:, j:j + 1],
                    pst[:, :],
                    Alu.mult,
                    Alu.add,
                )

            # ln(2S) in two 512-wide ACT passes
            for g in range(2):
                nc.scalar.activation(
                    LNS[:, g * 512:(g + 1) * 512],
                    S[:, g * 512:(g + 1) * 512],
                    Act.Ln,
                    scale=2.0,
                )

            # t2a = (lns - lx[n]) - ly[k]
            for j in range(NTILES):
                nc.vector.scalar_tensor_tensor(
                    T2A[:, j * 128:(j + 1) * 128],
                    LNS[:, j * 128:(j + 1) * 128],
                    LX[:, j:j + 1],
                    LYB[:, :],
                    Alu.subtract,
                    Alu.subtract,
                )

            # E = exp(-2 * t2a)
            for g in range(2):
                nc.scalar.activation(
                    E[:, g * 512:(g + 1) * 512],
                    T2A[:, g * 512:(g + 1) * 512],
                    Act.Exp,
                    scale=-2.0,
                )

            # dist = t2a - E ; stream colsums + output DMA per 256-chunk
            ncd_r = ncd_o[:, :].rearrange("(t p) d -> p t d", p=128)
            GPS1 = ps.tile([1, 128], f32, tag="GPS1")
            WRM = ps.tile([1, 256], f32, tag="WRM")
            # keep the PE clock ungated between the mm1 block and the
            # colsums (HAM gates an idle PE to half speed)
            for g in range(4):
                nc.tensor.matmul(
                    WRM[:, :], ones_col[:, :], S[:, g * 256:(g + 1) * 256],
                    start=True, stop=True,
                )
            for c in range(4):
                lo, hi = c * 256, (c + 1) * 256
                nc.vector.tensor_sub(DD[:, lo:hi], T2A[:, lo:hi], E[:, lo:hi])
                if not mask_ones:
                    for j in (2 * c, 2 * c + 1):
                        nc.vector.tensor_scalar(
                            NCD[:, j * 128:(j + 1) * 128],
                            DD[:, j * 128:(j + 1) * 128],
                            MK[:, j:j + 1],
                            None,
                            Alu.mult,
                        )
                out_eng = [nc.sync, nc.scalar, nc.sync, nc.scalar][c]
                out_eng.dma_start(
                    ncd_r[:, 2 * c:2 * c + 2, :],
                    NCD[:, lo:hi].rearrange("p (t d) -> p t d", d=128),
                )
                for h in range(2):
                    j = 2 * c + h
                    nc.tensor.matmul(
                        GPS1[:, :],
                        ones_col[:, :],
                        NCD[:, j * 128:(j + 1) * 128],
                        start=(j == 0),
                        stop=(j == NTILES - 1),
                    )

            # per-core partial [colsum(0:128) | mask-count(128)]; the host
            # adds the 8 partial vectors and divides (cross-core collectives
            # cost ~60-80us of barrier skew here for 516 bytes).
            ALLIN = sb.tile([1, 129], f32, tag="ALLIN")
            nc.vector.tensor_copy(ALLIN[:, 0:128], GPS1[:, :])
            if mask_ones:
                nc.vector.memset(ALLIN[:, 128:129], float(NS))
            else:
                MSp = ps.tile([1, NTILES], f32, tag="MSp")
                nc.tensor.matmul(MSp[:, :], ones_col[:, :], MK[:, :], start=True, stop=True)
                nc.vector.tensor_reduce(
                    ALLIN[:, 128:129], MSp[:, :], mybir.AxisListType.X, Alu.add
                )
            nc.sync.dma_start(g_o[:, :], ALLIN[:, :])

    bacc.get_activation_tables = _only_combined_set
    try:
        nc.compile()
    finally:
        bacc.get_activation_tables = _orig_tables
    return nc


def kernel(node_repr: np.ndarray, mask: np.ndarray, centroids: np.ndarray):
    from concourse.bass_utils import run_bass_kernel_spmd

    node_repr = np.ascontiguousarray(node_repr, dtype=np.float32)
    mask = np.ascontiguousarray(mask, dtype=np.float32)
    centroids = np.ascontiguousarray(centroids, dtype=np.float32)
    assert node_repr.shape == (N, D) and centroids.shape == (K, D)

    mask_ones = bool(np.all(mask == 1.0))
    key = ("nc", mask_ones)
    if key not in _CACHE:
        _CACHE[key] = _build(mask_ones)
    nc = _CACHE[key]

    ct = np.ascontiguousarray(centroids.T)
    in_maps = []
    for i in range(NCORES):
        shard = node_repr[i * NS:(i + 1) * NS]
        mshard = mask[i * NS:(i + 1) * NS, 0]
        in_maps.append({
            "nt": np.ascontiguousarray(shard.T),
            "nd": np.ascontiguousarray(shard),
            "ct": ct,
            "mkt": np.ascontiguousarray(mshard.reshape(NTILES, 128).T),
        })

    res = run_bass_kernel_spmd(nc, in_maps, list(range(NCORES)))
    outs = res.results

    ncd = np.concatenate([outs[i]["ncd"] for i in range(NCORES)], axis=0)
    parts = np.stack([outs[i]["gout"][0] for i in range(NCORES)])  # [8, 129]
    tot = parts.sum(axis=0, dtype=np.float32)
    graph = (tot[:K] / tot[K]).astype(np.float32)
    return (
        graph.reshape(1, K),
        ncd.reshape(1, N, K).astype(np.float32),
    )


# revision 43
# speedup vs baseline: 1.2779x; 1.0338x over previous
"""Trainium2 Bass kernel for CentroidDistance (Poincare ball, c=1).

Math (per node x, centroid y):
    x2 = |x|^2, y2 = |y|^2
    S    = den + e2 = -4<x,y> + (1+x2)(1+y2)     (den = 1-2xy+x2y2, e2 = |x-y|^2)
    num  = den - e2 = (1-x2)(1-y2)
    z    = S/num = cosh(dist)
    dist = acosh(z) = ln(2z) - E0 - 1.5*E0^2 - ...,  E0 = 1/(4 z^2) = exp(-2 ln 2z)
so with D = ln(2S) - ln(1-x2) - ln(1-y2):
    dist = D - exp(-2 D)            (error <= 1.5*E0^2 ~ 4e-5 for z >= 7)
Verified vs the fp64 oracle: absmax err 0.0075, 3x tighter than the fp32
reference's own envelope (0.023).

Sharding: data-parallel over node rows, 8 cores x 1024 nodes; centroids
replicated. Each core writes its node_centroid_dist shard plus a [1,129]
partial [colsum | mask-count]; the host adds the 8 partials and divides
(an on-device AllReduce of 516B costs ~60-80us of barrier skew here).
"""

import sys
import os

sys.path.insert(0, "/opt/trn_rl_repo")

import numpy as np

N, K, D = 8192, 128, 128
NCORES = 8
NS = N // NCORES          # 1024 nodes per core
NTILES = NS // 128        # 8 tiles of 128 nodes

_CACHE = {}


def _build(mask_ones: bool):
    """Build the SPMD Bass program (one NeuronCore; replicated on 8)."""
    import concourse.bass as bass
    import concourse.bacc as bacc
    import concourse.mybir as mybir
    from concourse import tile

    f32 = mybir.dt.float32
    f32r = mybir.dt.float32r
    bf16 = mybir.dt.bfloat16
    Alu = mybir.AluOpType
    Act = mybir.ActivationFunctionType

    # Force a single ACT table-set load: every activation here (Copy/Ln/Exp)
    # lives in natural_log_exp_and_others, but the per-func chooser would pick
    # exp_and_others for Exp and natural_log for Ln (2 x ~1.3us loads, one of
    # them mid-pipeline). Present every other set as empty during this build.
    _orig_tables = bacc.get_activation_tables

    def _only_combined_set(arch):
        t = _orig_tables(arch)
        return {
            name: (funcs if name == "natural_log_exp_and_others" else set())
            for name, funcs in t.items()
        }

    nc = bacc.Bacc(
        "TRN2", target_bir_lowering=False, debug=False, num_devices=1
    )

    nt = nc.dram_tensor("nt", [128, NS], f32, kind="ExternalInput")     # nodes^T (d, n)
    nd = nc.dram_tensor("nd", [NS, 128], f32, kind="ExternalInput")     # nodes (n, d)
    ct = nc.dram_tensor("ct", [128, 128], f32, kind="ExternalInput")    # centroids^T (d, k)
    mkt = nc.dram_tensor("mkt", [128, NTILES], f32, kind="ExternalInput")
    ncd_o = nc.dram_tensor("ncd", [NS, K], f32, kind="ExternalOutput")
    g_o = nc.dram_tensor("gout", [1, K + 1], f32, kind="ExternalOutput")

    with tile.TileContext(nc) as tc:
        with (
            tc.tile_pool(name="sb", bufs=1) as sb,
            tc.tile_pool(name="ps", bufs=1, space="PSUM") as ps,
            tc.tile_pool(name="psm", bufs=4, space="PSUM") as psm,
            tc.tile_pool(name="dram", bufs=1, space="DRAM") as dram,
        ):
            # ---------------- loads ----------------
            NT = sb.tile([128, NS], f32, tag="NT")
            ND = sb.tile([128, NS], f32, tag="ND")      # col j*128+q = nd[j*128+p, q]
            CT = sb.tile([128, 128], f32, tag="CT")
            MK = sb.tile([128, NTILES], f32, tag="MK")
            H = NS // 2
            nc.scalar.dma_start(CT[:, :], ct[:, :])
            nc.scalar.dma_start(MK[:, :], mkt[:, :])
            nd_r = nd[:, :].rearrange("(t p) d -> p t d", p=128)
            nd_s = ND[:, :].rearrange("p (t d) -> p t d", d=128)
            Q = NS // 4
            nt_eng = [nc.sync, nc.scalar, nc.sync, nc.scalar]
            for g in range(4):
                nc.gpsimd.dma_start(nd_s[:, g * 2:(g + 1) * 2, :], nd_r[:, g * 2:(g + 1) * 2, :])
                nt_eng[g].dma_start(NT[:, g * Q:(g + 1) * Q], nt[:, g * Q:(g + 1) * Q])

            # ---------------- constants / per-centroid setup ----------------
            ones_col = sb.tile([128, 1], f32, tag="ones_col")   # lhsT for col-sums
            nc.vector.memset(ones_col[:, :], 1.0)
            ones_row = sb.tile([1, 128], f32, tag="ones_row")   # lhsT for bcasts
            nc.vector.memset(ones_row[:, :], 1.0)

            NTB = sb.tile([128, NS], bf16, tag="NTB")
            for lo, hi in [(0, 256), (256, 512), (512, 768), (768, 1024)]:
                nc.scalar.activation(NTB[:, lo:hi], NT[:, lo:hi], Act.Copy)
            CTM4 = sb.tile([128, 128], bf16, tag="CTM4")
            nc.vector.tensor_scalar(CTM4[:, :], CT[:, :], -4.0, None, Alu.mult)

            with tc.high_priority():
                SQCT = sb.tile([128, 128], f32r, tag="SQCT")
                nc.vector.tensor_mul(SQCT[:, :], CT[:, :], CT[:, :])
                YRp = ps.tile([1, 128], f32, tag="setupps")         # y2 row
                nc.tensor.matmul(
                    YRp[:, :], ones_col[:, :].bitcast(f32r), SQCT[:, :],
                    start=True, stop=True,
                )

                BR2 = sb.tile([1, 256], f32r, tag="BR2")   # [1+y2 | 1-y2]
                nc.vector.tensor_scalar(BR2[:, 0:128], YRp[:, :], 1.0, None, Alu.add)
                nc.vector.tensor_scalar(BR2[:, 128:256], YRp[:, :], -1.0, 1.0, Alu.mult, Alu.add)

                BCp = ps.tile([128, 256], f32, tag="setupps")   # bcast of both rows
                nc.tensor.matmul(
                    BCp[:, :], ones_row[:, :].bitcast(f32r), BR2[:, :],
                    start=True, stop=True,
                )
                BOPY2 = sb.tile([128, 128], f32, tag="BOPY2")
                nc.vector.tensor_copy(BOPY2[:, :], BCp[:, 0:128])
                LYB = sb.tile([128, 128], f32, tag="LYB")       # ln(1-y2) bcast
                nc.scalar.activation(LYB[:, :], BCp[:, 128:256], Act.Ln)

            # ---------------- per-node setup: x2, 1+x2, ln(1-x2) ----------------
            X2 = sb.tile([128, NTILES], f32, tag="X2")
            SCR = sb.tile([128, 128], f32, tag="SCR")
            for j in range(NTILES):
                # SCR = (nd + 0) * nd = nd^2 ; accum_out = row-sum = x2
                nc.vector.scalar_tensor_tensor(
                    SCR[:, :],
                    ND[:, j * 128:(j + 1) * 128],
                    0.0,
                    ND[:, j * 128:(j + 1) * 128],
                    Alu.add,
                    Alu.mult,
                    accum_out=X2[:, j:j + 1],
                )
            OPX2 = sb.tile([128, NTILES], f32, tag="OPX2")
            for j in range(NTILES):
                nc.vector.tensor_scalar(
                    OPX2[:, j:j + 1], X2[:, j:j + 1], 1.0, None, Alu.add
                )
            LX = sb.tile([128, NTILES], f32, tag="LX")          # ln(1-x2)
            nc.scalar.activation(LX[:, :], X2[:, :], Act.Ln, bias=1.0, scale=-1.0)

            # ---------------- main grid ----------------
            S = sb.tile([128, NS], f32, tag="S")
            LNS = sb.tile([128, NS], f32, tag="LNS")
            T2B = sb.tile([128, NS], f32, tag="T2B")    # lns - lx
            T2A = sb.tile([128, NS], f32, tag="T2A")    # (lns - lx) - ly
            E = sb.tile([128, NS], f32, tag="E")
            DD = sb.tile([128, NS], f32r, tag="DD")
            NCD = DD if mask_ones else sb.tile([128, NS], f32r, tag="NCD")

            for j in range(NTILES):
                pst = psm.tile([128, 128], f32, tag="mmps")
                nc.tensor.matmul(
                    pst[:, :],
                    NTB[:, j * 128:(j + 1) * 128],
                    CTM4[:, :],
                    start=True,
                    stop=True,
                )
                # S = (1+y2)*(1+x2[n]) + (-4xy)
                nc.vector.scalar_tensor_tensor(
                    S[:, j * 128:(j + 1) * 128],
                    BOPY2[:, :],
                    OPX2[:, j:j + 1],
                    pst[:, :],
                    Alu.mult,
                    Alu.add,
                )

            # ln(2S) in two 512-wide ACT passes
            for g in range(2):
                nc.scalar.activation(
                    LNS[:, g * 512:(g + 1) * 512],
                    S[:, g * 512:(g + 1) * 512],
                    Act.Ln,
                    scale=2.0,
                )

            # t2a = (lns - lx[n]) - ly[k]
            for j in range(NTILES):
                nc.vector.scalar_tensor_tensor(
                    T2A[:, j * 128:(j + 1) * 128],
                    LNS[:, j * 128:(j + 1) * 128],
                    LX[:, j:j + 1],
                    LYB[:, :],
                    Alu.subtract,
                    Alu.subtract,
                )

            # E = exp(-2 * t2a)
            for g in range(2):
                nc.scalar.activation(
                    E[:, g * 512:(g + 1) * 512],
                    T2A[:, g * 512:(g + 1) * 512],
                    Act.Exp,
                    scale=-2.0,
                )

            # dist = t2a - E ; stream colsums + output DMA per 256-chunk
            ncd_r = ncd_o[:, :].rearrange("(t p) d -> p t d", p=128)
            GPS2 = ps.tile([1, 256], f32, tag="GPS2")
            WRM = ps.tile([1, 256], f32, tag="WRM")
            # keep the PE clock ungated between the mm1 block and the
            # colsums (HAM gates an idle PE to half speed)
            for g in range(4):
                nc.tensor.matmul(
                    WRM[:, :], ones_col[:, :], S[:, g * 256:(g + 1) * 256],
                    start=True, stop=True,
                )
            for c in range(4):
                lo, hi = c * 256, (c + 1) * 256
                nc.vector.tensor_sub(DD[:, lo:hi], T2A[:, lo:hi], E[:, lo:hi])
                if not mask_ones:
                    for j in (2 * c, 2 * c + 1):
                        nc.vector.tensor_scalar(
                            NCD[:, j * 128:(j + 1) * 128],
                            DD[:, j * 128:(j + 1) * 128],
                            MK[:, j:j + 1],
                            None,
                            Alu.mult,
                        )
                out_eng = [nc.sync, nc.scalar, nc.sync, nc.scalar][c]
                out_eng.dma_start(
                    ncd_r[:, 2 * c:2 * c + 2, :],
                    NCD[:, lo:hi].bitcast(f32).rearrange("p (t d) -> p t d", d=128),
                )
                nc.tensor.matmul(
                    GPS2[:, :],
                    ones_col[:, :].bitcast(f32r),
                    NCD[:, lo:hi],
                    start=(c == 0),
                    stop=(c == 3),
                )

            # per-core partial [colsum(0:128) | mask-count(128)]; the host
            # adds the 8 partial vectors and divides (cross-core collectives
            # cost ~60-80us of barrier skew here for 516 bytes).
            GH = sb.tile([1, 256], f32, tag="GH")
            nc.vector.tensor_copy(GH[:, :], GPS2[:, :])
            ALLIN = sb.tile([1, 129], f32, tag="ALLIN")
            nc.vector.tensor_add(ALLIN[:, 0:128], GH[:, 0:128], GH[:, 128:256])
            if mask_ones:
                nc.vector.memset(ALLIN[:, 128:129], float(NS))
            else:
                MSp = ps.tile([1, NTILES], f32, tag="MSp")
                nc.tensor.matmul(MSp[:, :], ones_col[:, :], MK[:, :], start=True, stop=True)
                nc.vector.tensor_reduce(
                    ALLIN[:, 128:129], MSp[:, :], mybir.AxisListType.X, Alu.add
                )
            nc.sync.dma_start(g_o[:, :], ALLIN[:, :])

    bacc.get_activation_tables = _only_combined_set
    try:
        nc.compile()
    finally:
        bacc.get_activation_tables = _orig_tables
    return nc


def kernel(node_repr: np.ndarray, mask: np.ndarray, centroids: np.ndarray):
    from concourse.bass_utils import run_bass_kernel_spmd

    node_repr = np.ascontiguousarray(node_repr, dtype=np.float32)
    mask = np.ascontiguousarray(mask, dtype=np.float32)
    centroids = np.ascontiguousarray(centroids, dtype=np.float32)
    assert node_repr.shape == (N, D) and centroids.shape == (K, D)

    mask_ones = bool(np.all(mask == 1.0))
    key = ("nc", mask_ones)
    if key not in _CACHE:
        _CACHE[key] = _build(mask_ones)
    nc = _CACHE[key]

    ct = np.ascontiguousarray(centroids.T)
    in_maps = []
    for i in range(NCORES):
        shard = node_repr[i * NS:(i + 1) * NS]
        mshard = mask[i * NS:(i + 1) * NS, 0]
        in_maps.append({
            "nt": np.ascontiguousarray(shard.T),
            "nd": np.ascontiguousarray(shard),
            "ct": ct,
            "mkt": np.ascontiguousarray(mshard.reshape(NTILES, 128).T),
        })

    res = run_bass_kernel_spmd(nc, in_maps, list(range(NCORES)))
    outs = res.results

    ncd = np.concatenate([outs[i]["ncd"] for i in range(NCORES)], axis=0)
    parts = np.stack([outs[i]["gout"][0] for i in range(NCORES)])  # [8, 129]
    tot = parts.sum(axis=0, dtype=np.float32)
    graph = (tot[:K] / tot[K]).astype(np.float32)
    return (
        graph.reshape(1, K),
        ncd.reshape(1, N, K).astype(np.float32),
    )
